# revision 1
# baseline (speedup 1.0000x reference)
"""Trainium2 Bass kernel for nn_Linear_6373731467798 (binarized dense layer).

Math (forward values only):
    act   = sign(x + bias)                      # +-1 (0 on exact zero)
    scale = mean(|weight|)
    w_eff = scale * sign(weight)
    out   = act @ w_eff.T = scale * (sign(x+bias) @ sign(weight).T)

Both matmul operands are in {-1, 0, +1}, exactly representable in bf16, and
PSUM accumulates in fp32, so a bf16 matmul is numerically exact; the single
fp32 `scale` multiply is applied on the way out of PSUM.

Sharding: data-parallel over 8 NeuronCores along the N=32768 batch dim
(4096 rows/core); bias and the binarized weight are replicated. No
collectives needed (forward only).

Per-core pipeline:
  prologue:  load W (4 MB) -> |W| row-sums (ACT accum) -> total via
             ones-matmul partition reduce -> scale[128,1];
             sign(W) bf16 -> PE-transpose -> W_T[i, o] bf16 resident in SBUF
  main loop (32 row-tiles of 128):
             DMA x-tile [128,1024] f32
             PE fp32-transpose (8x 128x128 blocks) -> PSUM
             ACT: sign(x_T + bias) fused (bias is per-partition after the
                  transpose) -> bf16 act_T blocks in SBUF
             16 bf16 matmuls (8 K-blocks x 2 output halves) -> PSUM
             DVE: out = psum * scale (per-partition scalar) -> SBUF
             DMA out-tile
"""

import sys

for _p in ("/opt/trn_rl_repo",):
    if _p not in sys.path:
        sys.path.insert(0, _p)

import numpy as np

import concourse.bass as bass
import concourse.tile as tile
from concourse import bacc, mybir

N = 32768
D = 1024
NCORES = 8
NSHARD = N // NCORES  # 4096
P = 128
NB = D // P  # 8 contraction blocks
GN = 4  # row-tiles per DMA group (512 rows / 2 MB per DMA)
NGROUP = NSHARD // (GN * P)  # 8 groups
F32 = mybir.dt.float32
BF16 = mybir.dt.bfloat16


def build_program(num_cores: int = NCORES) -> bass.Bass:
    from contextlib import ExitStack

    from concourse.masks import make_identity

    nc = bacc.Bacc(
        "TRN2",
        target_bir_lowering=False,
        debug=False,
        enable_asserts=True,
        num_devices=num_cores,
    )

    x_ap = nc.dram_tensor("x", [NSHARD, D], F32, kind="ExternalInput").ap()
    b_ap = nc.dram_tensor("bias", [D], F32, kind="ExternalInput").ap()
    w_ap = nc.dram_tensor("weight", [D, D], F32, kind="ExternalInput").ap()
    o_ap = nc.dram_tensor("out", [NSHARD, D], F32, kind="ExternalOutput").ap()

    with tile.TileContext(nc) as tc, ExitStack() as ctx:
        const = ctx.enter_context(tc.tile_pool(name="const", bufs=1))
        wt_pool = ctx.enter_context(tc.tile_pool(name="wT", bufs=1))
        psum_aux = ctx.enter_context(tc.tile_pool(name="psum_aux", bufs=2, space="PSUM"))

        ident_f = const.tile([P, P], F32, tag="ident_f")
        make_identity(nc, ident_f[:])
        ident_b = const.tile([P, P], BF16, tag="ident_b")
        make_identity(nc, ident_b[:])

        # bias_sb[p, b] = bias[b*128 + p]  (per-partition bias per i-block)
        bias_sb = const.tile([P, NB], F32, tag="bias")
        nc.sync.dma_start(out=bias_sb[:], in_=b_ap.rearrange("(b p) -> p b", p=P))

        # ---- weight prologue ----
        with tc.tile_pool(name="wstage", bufs=1) as wstage:
            # wfull[p, t, :] = weight[t*128 + p, :]
            wfull = wstage.tile([P, NB, D], F32, tag="wfull")
            nc.sync.dma_start(
                out=wfull[:], in_=w_ap.rearrange("(t p) i -> p t i", p=P)
            )

            # |w| accumulated along free dim, per o-block -> asum[:, t]
            asum = const.tile([P, NB], F32, tag="asum")
            wsg = wstage.tile([P, NB, D], BF16, tag="wsg")
            wscr = wstage.tile([P, NB, D], BF16, tag="wscr")
            for t in range(NB):
                nc.scalar.activation(
                    wscr[:, t, :],
                    wfull[:, t, :],
                    mybir.ActivationFunctionType.Abs,
                    accum_out=asum[:, t : t + 1],
                )
                nc.scalar.sign(wsg[:, t, :], wfull[:, t, :])

            # reduce asum over its NB columns (Abs is identity on >=0 values)
            colsum = const.tile([P, 1], F32, tag="colsum")
            ascr = const.tile([P, NB], BF16, tag="ascr")
            nc.scalar.activation(
                ascr[:],
                asum[:],
                mybir.ActivationFunctionType.Abs,
                accum_out=colsum[:],
            )
            # partition reduce + broadcast via ones-matmuls
            ones_col = const.tile([P, 1], F32, tag="ones_col")
            nc.vector.memset(ones_col[:], 1.0)
            ones_row = const.tile([1, P], F32, tag="ones_row")
            nc.vector.memset(ones_row[:], 1.0)
            tot_ps = psum_aux.tile([1, 1], F32, tag="aux")
            nc.tensor.matmul(tot_ps[:], ones_col[:], colsum[:], start=True, stop=True)
            tot_sb = const.tile([1, 1], F32, tag="tot")
            nc.vector.tensor_copy(tot_sb[:], tot_ps[:])
            bcast_ps = psum_aux.tile([P, 1], F32, tag="aux")
            nc.tensor.matmul(bcast_ps[:], ones_row[:], tot_sb[:], start=True, stop=True)
            scale_sb = const.tile([P, 1], F32, tag="scale")
            nc.vector.tensor_scalar_mul(scale_sb[:], bcast_ps[:], 1.0 / (D * D))

            # W_T[p2, b, o] = sign(weight)[o, b*128+p2]   (bf16, resident)
            w_t = wt_pool.tile([P, NB, D], BF16, tag="wT")
            for b in range(NB):
                pwt = psum_aux.tile([P, D], BF16, tag="aux")
                for t in range(NB):
                    nc.tensor.transpose(
                        pwt[:, t * P : (t + 1) * P],
                        wsg[:, t, b * P : (b + 1) * P],
                        ident_b[:],
                    )
                nc.vector.tensor_copy(w_t[:, b, :], pwt[:])

        # ---- main loop ----
        xpool = ctx.enter_context(tc.tile_pool(name="x", bufs=3))
        apool = ctx.enter_context(tc.tile_pool(name="actT", bufs=4))
        opool = ctx.enter_context(tc.tile_pool(name="o", bufs=3))
        psum_x = ctx.enter_context(tc.tile_pool(name="psum_x", bufs=2, space="PSUM"))
        psum_mm = ctx.enter_context(tc.tile_pool(name="psum_mm", bufs=2, space="PSUM"))

        for g in range(NGROUP):
            rows = slice(g * GN * P, (g + 1) * GN * P)
            x_sb = xpool.tile([P, GN, D], F32, tag="x")
            nc.sync.dma_start(
                out=x_sb[:], in_=x_ap[rows, :].rearrange("(a p) i -> p a i", p=P)
            )
            o_sb = opool.tile([P, GN, D], F32, tag="o")
            for j in range(GN):
                act_t = apool.tile([P, NB, P], BF16, tag="actT")
                for half in range(2):
                    pt = psum_x.tile([P, 512], F32, tag="xtr")
                    for q in range(4):
                        b = half * 4 + q
                        nc.tensor.transpose(
                            pt[:, q * P : (q + 1) * P],
                            x_sb[:, j, b * P : (b + 1) * P],
                            ident_f[:],
                        )
                    for q in range(4):
                        b = half * 4 + q
                        nc.scalar.sign(
                            act_t[:, b, :],
                            pt[:, q * P : (q + 1) * P],
                            bias=bias_sb[:, b : b + 1],
                        )
                po = psum_mm.tile([P, D], F32, tag="mm")
                for b in range(NB):
                    for h2 in range(2):
                        nc.tensor.matmul(
                            po[:, h2 * 512 : (h2 + 1) * 512],
                            act_t[:, b, :],
                            w_t[:, b, h2 * 512 : (h2 + 1) * 512],
                            start=(b == 0),
                            stop=(b == NB - 1),
                        )
                for h2 in range(2):
                    nc.vector.tensor_scalar_mul(
                        o_sb[:, j, h2 * 512 : (h2 + 1) * 512],
                        po[:, h2 * 512 : (h2 + 1) * 512],
                        scale_sb[:],
                    )
            nc.sync.dma_start(
                out=o_ap[rows, :].rearrange("(a p) i -> p a i", p=P), in_=o_sb[:]
            )

    nc.compile()
    return nc


_PROGRAM_CACHE: dict[int, bass.Bass] = {}


def _get_program(num_cores: int = NCORES) -> bass.Bass:
    if num_cores not in _PROGRAM_CACHE:
        _PROGRAM_CACHE[num_cores] = build_program(num_cores)
    return _PROGRAM_CACHE[num_cores]


def kernel(x: np.ndarray, bias: np.ndarray, weight: np.ndarray) -> np.ndarray:
    from concourse.bass_utils import run_bass_kernel_spmd

    x = np.ascontiguousarray(np.asarray(x, dtype=np.float32))
    bias = np.ascontiguousarray(np.asarray(bias, dtype=np.float32))
    weight = np.ascontiguousarray(np.asarray(weight, dtype=np.float32))
    assert x.shape == (N, D) and bias.shape == (D,) and weight.shape == (D, D)

    nc = _get_program(NCORES)
    in_maps = [
        {"x": x[c * NSHARD : (c + 1) * NSHARD], "bias": bias, "weight": weight}
        for c in range(NCORES)
    ]
    res = run_bass_kernel_spmd(nc, in_maps, list(range(NCORES)))
    return np.concatenate([res.results[c]["out"] for c in range(NCORES)], axis=0)
